# revision 42
# baseline (speedup 1.0000x reference)
"""Multi-head causal self-attention (B=2, S=2048, D=1024, H=16) on 8 TRN2 cores.

Sharding: head-parallel attention + token-parallel output projection.
Core c owns heads {2c, 2c+1} (128 of the 1024 qkv dims, both batches) for
QKV/attention; an AllToAll then redistributes the normalized context so
core c owns a token block with ALL 1024 ctx dims and computes the full
output projection for its tokens with a replicated Wo.

Schedule (the attention exp on the scalar engine is the per-batch
bottleneck at ~50us/batch; PE work is software-pipelined into its shadow):

  B(b0): QKV proj batch 0. V computed directly in [token, dim] layout
         (x-tile stationary) - no PE transposes.
  D(b0): scores/exp/ctx, interleaved with B(b1) matmuls as PE fillers so
         the tensor engine stays dense (HAM stays warm) while ACT exps.
         After each half of the sequence: normalize (DVE reciprocal) and
         fire a half-batch AllToAll ([1024, 128] bf16) immediately.
  D(b1): same, with G(b0) output-projection matmuls as PE fillers.
  G(b1): final output projection; only the last half-A2A + 64 small
         matmuls are exposed at the tail.

All matmul operands bf16 (fp32r runs fp32_mode=HIGH ~3x slower on this
HW); PSUM accumulation fp32; softmax stats fp32. l = sum(exp) comes free
from a ones-column in V_aug (row 64 of ctx PSUM).
"""

import sys

for p in ("/opt/trn_rl_repo", "/root/.axon_site/_ro/trn_rl_repo"):
    if p not in sys.path:
        sys.path.insert(0, p)

import numpy as np
import ml_dtypes

import bass_rust
import concourse.bass as bass
import concourse.mybir as mybir
from concourse.bass_utils import run_bass_kernel_spmd
from concourse.tile import TileContext

B, S, D = 2, 2048, 1024
H, DH = 16, 64
T = B * S              # 4096 tokens
NC = 8                 # cores
HG = D // NC           # 128 qkv dims per core (2 heads)
HB = S // 2            # 1024 tokens per half-batch A2A
TB = HB // NC          # 128 tokens per core per half-batch
KT_D = D // 128        # 8 contraction tiles over d_model
INV_SCALE = 1.0 / float(np.sqrt(DH))  # 1/8
F32 = mybir.dt.float32
BF16 = mybir.dt.bfloat16
BF16NP = ml_dtypes.bfloat16


def _split_waits(nc, max_waits=1):
    """This walrus build accepts one sync-wait per instruction; Tile sometimes
    emits more. Split extras into preceding NoOps on the same engine."""
    n = 0
    for f in nc.m.functions:
        for bb in f.blocks:
            out = []
            for inst in bb.instructions:
                si = getattr(inst, "sync_info", None)
                if si is not None and si.on_wait and len(si.on_wait) > max_waits:
                    waits = list(si.on_wait)
                    head, rest = waits[:-max_waits], waits[-max_waits:]
                    k = 0
                    while head:
                        chunk, head = head[:max_waits], head[max_waits:]
                        out.append(mybir.InstNoOp(
                            name=f"{inst.name}-wsplit-{k}", ins=[], outs=[],
                            engine=inst.engine,
                            sync_info=bass_rust.SyncInfo(on_wait=chunk, on_update=[]),
                        ))
                        k += 1
                    si.on_wait = rest
                    n += 1
                out.append(inst)
            bb.instructions = out
    return n


class FillerQueue:
    """Ordered PE work units pulled into dependency-stall windows.

    Units are (cost_ns, group_id, fn). pull(budget) emits whole units until
    the budget is spent. finish_group() emits the rest of the group the last
    emitted unit belongs to - REQUIRED before any other user touches a PSUM
    ring the in-progress group holds (in-order engine queues would deadlock
    on a ring slot whose release is queued behind the waiter).
    """

    def __init__(self, units):
        self.units = list(units)
        self.i = 0
        self.cur_group = None

    def pull(self, budget):
        while self.i < len(self.units) and budget > 0:
            cost, grp, fn = self.units[self.i]
            fn()
            self.cur_group = grp
            budget -= cost
            self.i += 1

    def finish_group(self):
        if self.cur_group is None:
            return
        while self.i < len(self.units):
            cost, grp, fn = self.units[self.i]
            if grp != self.cur_group:
                break
            fn()
            self.i += 1
        self.cur_group = None

    def drain(self):
        while self.i < len(self.units):
            self.units[self.i][2]()
            self.i += 1
        self.cur_group = None


def build_module():
    nc = bass.Bass()

    xT = nc.dram_tensor("xT", [D, T], BF16, kind="ExternalInput")
    # weights pre-rearranged on host to [p, kt*n] so the SBUF loads are one
    # contiguous run per partition (128 big descriptors, not 1024+ tiny ones)
    wq = nc.dram_tensor("wq", [128, KT_D * HG], BF16, kind="ExternalInput")
    wk = nc.dram_tensor("wk", [128, KT_D * HG], BF16, kind="ExternalInput")
    wv = nc.dram_tensor("wv", [128, KT_D * HG], BF16, kind="ExternalInput")
    wo = nc.dram_tensor("wo", [128, KT_D * D], BF16, kind="ExternalInput")  # FULL Wo
    bq = nc.dram_tensor("bq", [HG, 1], F32, kind="ExternalInput")
    bk = nc.dram_tensor("bk", [HG, 1], F32, kind="ExternalInput")
    bvr = nc.dram_tensor("bvr", [1, HG], BF16, kind="ExternalInput")  # bv as a row
    bo = nc.dram_tensor("bo", [128, KT_D], F32, kind="ExternalInput")  # bo[ot*128+p] = [p, ot]
    yT = nc.dram_tensor("yT", [D, 2 * S // NC], F32, kind="ExternalOutput")

    # AllToAll buffers, one pair per (batch, seq-half). Row-block j of
    # a2a_in = this core's 128 ctx dims for token block j of that half;
    # after A2A, row-block i of a2a_out = core i's dims for MY block.
    a2a_in = [nc.dram_tensor(f"a2ai{i}", [D, TB], BF16) for i in range(4)]
    a2a_out = [nc.dram_tensor(f"a2ao{i}", [D, TB], BF16) for i in range(4)]

    with TileContext(nc) as tc:
        with tc.tile_pool(name="persist", bufs=1) as pp:
            w_sb = {}
            for name, dram in (("wq", wq), ("wk", wk), ("wv", wv)):
                t = pp.tile([128, KT_D, HG], BF16, name=f"{name}_sb", tag=f"{name}_sb")
                nc.sync.dma_start(out=t[:], in_=dram[:].rearrange("p (kt n) -> p kt n", n=HG))
                w_sb[name] = t
            b_sb = {}
            for name, dram, cols in (("bq", bq, 1), ("bk", bk, 1), ("bo", bo, KT_D)):
                t = pp.tile([128, cols], F32, name=f"{name}_sb", tag=f"{name}_sb")
                nc.sync.dma_start(out=t[:], in_=dram[:])
                b_sb[name] = t
            bvr_sb = pp.tile([65, HG], BF16, name="bvr_sb", tag="bvr_sb")
            nc.sync.dma_start(out=bvr_sb[64:65, :], in_=bvr[:])

            # multiplicative causal mask for a diagonal 128x128 scores^T tile:
            # tri01[r, c] = 1 where r <= c (k <= q), else 0
            tri01 = pp.tile([128, 128], F32, name="tri01", tag="tri01")
            nc.gpsimd.memset(tri01[:], 1.0)
            nc.gpsimd.affine_select(
                out=tri01[:], in_=tri01[:],
                compare_op=mybir.AluOpType.is_ge, fill=0.0,
                base=0, pattern=[[1, 128]], channel_multiplier=-1,
            )
            tri01b = pp.tile([128, 128], BF16, name="tri01b", tag="tri01b")
            nc.vector.tensor_copy(tri01b[:], tri01[:])
            ones_f = pp.tile([65, 128], F32, name="ones_f", tag="ones_f")
            nc.vector.memset(ones_f[:], 1.0)
            ones_b = pp.tile([65, 128], BF16, name="ones_b", tag="ones_b")
            nc.vector.tensor_copy(ones_b[:], ones_f[:])
            ones128 = pp.tile([128, 64], F32, name="ones128", tag="ones128")
            nc.vector.memset(ones128[:], 1.0)

            qkvT = {}
            for name in ("qT", "kT"):
                qkvT[name] = [pp.tile([128, S], BF16, name=f"{name}{b}", tag=f"{name}{b}")
                              for b in range(B)]
            vaug = pp.tile([128, B * 2, S // 128, DH + 1], BF16, name="vaug", tag="vaug")
            nc.vector.tensor_copy(vaug[:, :, :, DH:DH + 1], ones128[:, :])
            ctxu = pp.tile([128, B * 2, S], F32, name="ctxu", tag="ctxu")
            ctxn = [pp.tile([128, S], BF16, name=f"ctxn{b}", tag=f"ctxn{b}")
                    for b in range(B)]
            bv_bc = pp.tile([128, HG], F32, name="bv_bc", tag="bv_bc")
            r_f = [pp.tile([65, S], F32, name=f"r_f{h}", tag=f"r_f{h}") for h in range(2)]
            r_b = [pp.tile([65, S], BF16, name=f"r_b{h}", tag=f"r_b{h}") for h in range(2)]

            with (
                tc.tile_pool(name="xt_pool", bufs=24) as xt_pool,
                tc.tile_pool(name="psS", bufs=2, space="PSUM") as psS_pool,
                tc.tile_pool(name="psC", bufs=2, space="PSUM") as psC_pool,
                tc.tile_pool(name="aux", bufs=2, space="PSUM") as aux_pool,
                tc.tile_pool(name="exp_pool", bufs=4) as exp_pool,
                tc.tile_pool(name="spool", bufs=2) as spool,
                tc.tile_pool(name="gpool", bufs=2) as gpool,
                tc.tile_pool(name="ypool", bufs=4) as ypool,
            ):
                def aux_tile():
                    return aux_pool.tile([128, 512], F32, name="aux", tag="aux")

                # bv broadcast [128, HG]: every row = bv (PE outer product)
                psb = aux_tile()
                nc.tensor.matmul(psb[:, 0:HG], ones_b[64:65, :], bvr_sb[64:65, :],
                                 start=True, stop=True)
                nc.vector.tensor_copy(bv_bc[:], psb[:, 0:HG])

                # ---------- stage B as interleavable units ----------
                def make_B_units(b, tqs=(0, 1)):
                    units = []
                    for tq in tqs:
                        t0 = tq * 1024
                        xts = {}

                        def dma_unit(b=b, tq=tq, t0=t0, xts=xts):
                            def fn(kt):
                                def go():
                                    xt = xt_pool.tile([128, 1024], BF16, name="xt", tag="xt")
                                    nc.sync.dma_start(
                                        out=xt[:],
                                        in_=xT[kt * 128:(kt + 1) * 128,
                                               b * S + t0: b * S + t0 + 1024])
                                    xts[kt] = xt
                                return go
                            return fn
                        mk_dma = dma_unit()
                        for kt in range(KT_D):
                            units.append((50, f"b{b}t{tq}dma", mk_dma(kt)))

                        for proj, bname in (("wq", "bq"), ("wk", "bk")):
                            st = {}

                            def mk_mm(proj=proj, st=st, xts=xts):
                                def fn(kt):
                                    def go():
                                        if kt == 0:
                                            st["ps0"] = aux_tile()
                                            st["ps1"] = aux_tile()
                                        for nch in range(2):
                                            nc.tensor.matmul(
                                                st["ps0" if nch == 0 else "ps1"][:],
                                                w_sb[proj][:, kt, :],
                                                xts[kt][:, nch * 512:(nch + 1) * 512],
                                                start=(kt == 0), stop=(kt == KT_D - 1),
                                                skip_group_check=True,
                                            )
                                    return go
                                return fn
                            mm = mk_mm()
                            for kt in range(KT_D):
                                units.append((440, f"b{b}t{tq}{proj}", mm(kt)))

                            def mk_bias(proj=proj, bname=bname, st=st, b=b, t0=t0):
                                def go():
                                    dname = "qT" if proj == "wq" else "kT"
                                    for nch in range(2):
                                        nc.vector.tensor_scalar_add(
                                            out=qkvT[dname][b][:, t0 + nch * 512:
                                                               t0 + (nch + 1) * 512],
                                            in0=st["ps0" if nch == 0 else "ps1"][:],
                                            scalar1=b_sb[bname][:, 0:1],
                                        )
                                return go
                            units.append((0, f"b{b}t{tq}{proj}", mk_bias()))

                        # V direct: [tok, dim] via x-tile stationary
                        def mk_v(b=b, tq=tq, xts=xts):
                            def fn(tt):
                                def go():
                                    psv = aux_tile()
                                    for kt in range(KT_D):
                                        nc.tensor.matmul(
                                            psv[:, 0:HG],
                                            xts[kt][:, tt * 128:(tt + 1) * 128],
                                            w_sb["wv"][:, kt, :],
                                            start=(kt == 0), stop=(kt == KT_D - 1),
                                            skip_group_check=True,
                                        )
                                    for h in range(2):
                                        nc.vector.tensor_add(
                                            out=vaug[:, b * 2 + h, tq * 8 + tt, 0:DH],
                                            in0=psv[:, h * DH:(h + 1) * DH],
                                            in1=bv_bc[:, h * DH:(h + 1) * DH],
                                        )
                                return go
                            return fn
                            # (unreachable)
                        mkv = mk_v()
                        for tt in range(8):
                            units.append((940, f"b{b}t{tq}v{tt}", mkv(tt)))
                    return units

                # ---------- stage G units (output projection, per batch) ----------
                def make_G_units(b):
                    units = []
                    st = {}

                    def dma_fn(st=st, b=b):
                        def go():
                            ctxg = gpool.tile([128, KT_D, 2 * TB], BF16, name="ctxg",
                                              tag="ctxg")
                            for half in range(2):
                                nc.sync.dma_start(
                                    out=ctxg[:, :, half * TB:(half + 1) * TB],
                                    in_=a2a_out[b * 2 + half][:].rearrange(
                                        "(kt p) n -> p kt n", p=128))
                            st["ctxg"] = ctxg
                        return go
                    units.append((100, f"g{b}d", dma_fn()))

                    def mk_ot(st=st, b=b):
                        def fn(ot):
                            def go():
                                psg = aux_tile()
                                for kt in range(KT_D):
                                    nc.tensor.matmul(
                                        psg[:, 0:2 * TB],
                                        wo_sb[:, kt, ot * 128:(ot + 1) * 128],
                                        st["ctxg"][:, kt, :],
                                        start=(kt == 0), stop=(kt == KT_D - 1),
                                        skip_group_check=True,
                                    )
                                yo = ypool.tile([128, 2 * TB], F32, name="yo", tag="yo")
                                nc.vector.tensor_scalar_add(
                                    out=yo[:], in0=psg[:, 0:2 * TB],
                                    scalar1=b_sb["bo"][:, ot:ot + 1])
                                nc.sync.dma_start(
                                    out=yT[ot * 128:(ot + 1) * 128,
                                           b * 2 * TB:(b + 1) * 2 * TB],
                                    in_=yo[:])
                            return go
                        return fn
                    mko = mk_ot()
                    for ot in range(KT_D):
                        units.append((1100, f"g{b}o{ot}", mko(ot)))
                    return units

                # G for batch 1, one seq-half at a time (N=128 matmuls):
                # half 0 depends only on A2A1a (complete ~40us earlier) and
                # runs while A2A1b is still in flight, so only half 1's ~7us
                # of matmuls are exposed at the very tail.
                def make_G1_units(half):
                    units = []
                    st = {}

                    def dma_fn(st=st, half=half):
                        def go():
                            ctxg = gpool.tile([128, KT_D, TB], BF16, name="ctxg1",
                                              tag="ctxg1")
                            nc.sync.dma_start(
                                out=ctxg[:],
                                in_=a2a_out[2 + half][:].rearrange(
                                    "(kt p) n -> p kt n", p=128))
                            st["ctxg"] = ctxg
                        return go
                    units.append((100, f"G1{half}d", dma_fn()))

                    def mk_ot(st=st, half=half):
                        def fn(ot):
                            def go():
                                psg = aux_tile()
                                for kt in range(KT_D):
                                    nc.tensor.matmul(
                                        psg[:, 0:TB],
                                        wo_sb[:, kt, ot * 128:(ot + 1) * 128],
                                        st["ctxg"][:, kt, :],
                                        start=(kt == 0), stop=(kt == KT_D - 1),
                                        skip_group_check=True,
                                    )
                                yo = ypool.tile([128, TB], F32, name="yo1", tag="yo1")
                                nc.vector.tensor_scalar_add(
                                    out=yo[:], in0=psg[:, 0:TB],
                                    scalar1=b_sb["bo"][:, ot:ot + 1])
                                nc.sync.dma_start(
                                    out=yT[ot * 128:(ot + 1) * 128,
                                           (2 + half) * TB:(3 + half) * TB],
                                    in_=yo[:])
                            return go
                        return fn
                    mko = mk_ot()
                    for ot in range(KT_D):
                        units.append((600, f"G1{half}o{ot}", mko(ot)))
                    return units

                # ---------- stage D (+E/F/A2A) with fillers ----------
                def emit_D(b, fillers):
                    for half in range(2):
                        filler = fillers[half]
                        for h in range(2):
                            pr = b * 2 + h
                            qT_h = qkvT["qT"][b][h * DH:(h + 1) * DH, :]
                            kT_h = qkvT["kT"][b][h * DH:(h + 1) * DH, :]
                            for qc in (2 * half, 2 * half + 1):
                                q0 = qc * 512
                                n_kt = q0 // 128 + 4
                                ps_ctx = psC_pool.tile([128, 512], F32, name="ps_ctx",
                                                       tag="ps_ctx")
                                for kg in range(n_kt // 2):
                                    ka, kb = 2 * kg, 2 * kg + 1
                                    offa = max(0, ka * 128 - q0)
                                    offb = max(0, kb * 128 - q0)
                                    ps_s = psS_pool.tile([128, 1024], F32, name="ps_s",
                                                         tag="ps_s")
                                    nc.tensor.matmul(
                                        ps_s[:, offa:512],
                                        kT_h[:, ka * 128:(ka + 1) * 128],
                                        qT_h[:, q0 + offa:q0 + 512],
                                        start=True, stop=True,
                                    )
                                    nc.tensor.matmul(
                                        ps_s[:, 512 + offb:1024],
                                        kT_h[:, kb * 128:(kb + 1) * 128],
                                        qT_h[:, q0 + offb:q0 + 512],
                                        start=True, stop=True,
                                    )
                                    ex = exp_pool.tile([128, 1024], BF16, name="ex",
                                                       tag="ex")
                                    # one exp over both halves; the gap
                                    # [512:512+offb) is stale-but-finite and
                                    # never read by the ctx matmuls.
                                    nc.scalar.activation(
                                        out=ex[:, offa:1024], in_=ps_s[:, offa:1024],
                                        func=mybir.ActivationFunctionType.Exp,
                                        scale=INV_SCALE,
                                    )
                                    if ka * 128 >= q0:
                                        nc.vector.tensor_mul(
                                            out=ex[:, offa:offa + 128],
                                            in0=ex[:, offa:offa + 128],
                                            in1=tri01b[:],
                                        )
                                    if kb * 128 >= q0:
                                        nc.vector.tensor_mul(
                                            out=ex[:, 512 + offb:512 + offb + 128],
                                            in0=ex[:, 512 + offb:512 + offb + 128],
                                            in1=tri01b[:],
                                        )
                                    nc.tensor.matmul(
                                        ps_ctx[0:DH + 1, offa:512],
                                        vaug[:, pr, ka, :],
                                        ex[:, offa:512],
                                        start=(ka == 0), stop=False,
                                        skip_group_check=True,
                                    )
                                    nc.tensor.matmul(
                                        ps_ctx[0:DH + 1, offb:512],
                                        vaug[:, pr, kb, :],
                                        ex[:, 512 + offb:1024],
                                        start=False, stop=(kb == n_kt - 1),
                                        skip_group_check=True,
                                    )
                                    # fill the PE's exp-wait window
                                    act_ns = (1024 - offa + 352) * 0.96
                                    pe_ns = ((512 - offa) + (512 - offb)) * 2 * 0.42 + 250
                                    filler.pull(max(0, act_ns - pe_ns))
                                nc.vector.tensor_copy(
                                    ctxu[0:DH + 1, pr, q0:q0 + 512],
                                    ps_ctx[0:DH + 1, :],
                                )
                        # ---- E: r = 1/l = exp(-ln(l)) on ACT (the DVE
                        # reciprocal is ~6.4ns/elem on a single-partition row,
                        # and SBUF->SBUF spread DMAs wedge the collective
                        # stream - measured, not theorized).
                        filler.finish_group()  # aux ring about to be reused
                        hs = slice(half * HB, (half + 1) * HB)
                        for h in range(2):
                            pr = b * 2 + h
                            nc.scalar.activation(
                                out=r_f[h][64:65, hs], in_=ctxu[64:65, pr, hs],
                                func=mybir.ActivationFunctionType.Ln)
                            nc.scalar.activation(
                                out=r_b[h][64:65, hs], in_=r_f[h][64:65, hs],
                                func=mybir.ActivationFunctionType.Exp, scale=-1.0)
                        for qc in (2 * half, 2 * half + 1):
                            q0 = qc * 512
                            for h in range(2):
                                pr = b * 2 + h
                                bc = aux_tile()
                                nc.tensor.matmul(
                                    bc[0:DH, :],
                                    ones_b[64:65, 0:DH],
                                    r_b[h][64:65, q0:q0 + 512],
                                    start=True, stop=True,
                                )
                                nc.vector.tensor_mul(
                                    out=ctxn[b][h * DH:(h + 1) * DH, q0:q0 + 512],
                                    in0=ctxu[0:DH, pr, q0:q0 + 512],
                                    in1=bc[0:DH, :],
                                )
                        idx = b * 2 + half
                        for j in range(NC):
                            nc.sync.dma_start(
                                out=a2a_in[idx][j * HG:(j + 1) * HG, :],
                                in_=ctxn[b][:, half * HB + j * TB:
                                            half * HB + (j + 1) * TB])
                        nc.gpsimd.collective_compute(
                            "AllToAll",
                            mybir.AluOpType.bypass,
                            ins=[a2a_in[idx][:]],
                            outs=[a2a_out[idx][:]],
                            replica_groups=[list(range(NC))],
                        )

                # ---------- emission ----------
                FillerQueue(make_B_units(0)).drain()
                # Wo arrives while attention runs; emitted after B so the
                # 2MB transfer doesn't delay the first x tiles.
                wo_sb = pp.tile([128, KT_D, D], BF16, name="wo_sb", tag="wo_sb")
                nc.sync.dma_start(
                    out=wo_sb[:], in_=wo[:].rearrange("p (kt n) -> p kt n", n=D))

                # B(b1) fills D(b0)'s exp-wait windows, one tq per seq-half so
                # late-queue units can't be pulled before their inputs exist.
                b1_fill = [FillerQueue(make_B_units(1, (0,))),
                           FillerQueue(make_B_units(1, (1,)))]
                emit_D(0, b1_fill)
                for f in b1_fill:
                    f.drain()  # leftovers are D(b1) prerequisites
                # G(b0) fills D(b1)'s SECOND half only - by then both of
                # batch 0's A2As are certainly complete, so the ctxg DMAs
                # never block the sync queue.
                g_fill = [FillerQueue([]), FillerQueue(make_G_units(0))]
                emit_D(1, g_fill)
                for f in g_fill:
                    f.drain()
                # G(b1)-half0 overlaps A2A1b's flight; half1 is the only
                # exposed tail work.
                FillerQueue(make_G1_units(0)).drain()
                FillerQueue(make_G1_units(1)).drain()

    _split_waits(nc)
    return nc


def kernel(x, mask, Wq, bq, Wk, bk, Wv, bv, Wo, bo, trace=False):
    x = np.asarray(x, dtype=np.float32).reshape(T, D)
    xT = np.ascontiguousarray(x.T).astype(BF16NP)

    def prearrange(w):
        # [D, n] -> [p, kt*n]: row p holds the kt-major sequence of chunks
        w = np.asarray(w, np.float32)
        n = w.shape[1]
        return np.ascontiguousarray(
            w.reshape(KT_D, 128, n).transpose(1, 0, 2).reshape(128, KT_D * n)
        ).astype(BF16NP)

    wo_full = prearrange(Wo)
    bo_t = np.ascontiguousarray(
        np.asarray(bo, np.float32).reshape(KT_D, 128).T)  # [p, ot]
    in_maps = []
    for c in range(NC):
        sl = slice(c * HG, (c + 1) * HG)
        in_maps.append({
            "xT": xT,
            "wq": prearrange(np.asarray(Wq, np.float32)[:, sl]),
            "wk": prearrange(np.asarray(Wk, np.float32)[:, sl]),
            "wv": prearrange(np.asarray(Wv, np.float32)[:, sl]),
            "wo": wo_full,
            "bq": np.ascontiguousarray(np.asarray(bq, np.float32)[sl].reshape(HG, 1)),
            "bk": np.ascontiguousarray(np.asarray(bk, np.float32)[sl].reshape(HG, 1)),
            "bvr": np.ascontiguousarray(np.asarray(bv, np.float32)[sl].reshape(1, HG)).astype(BF16NP),
            "bo": bo_t,
        })
    nc = build_module()
    res = run_bass_kernel_spmd(nc, in_maps, core_ids=list(range(NC)), trace=trace)
    out = np.empty((B, S, D), dtype=np.float32)
    for c in range(NC):
        yt = np.asarray(res.results[c]["yT"], dtype=np.float32)  # [1024, 512]
        for b in range(B):
            for half in range(2):
                t0 = half * HB + c * TB
                out[b, t0:t0 + TB, :] = yt[:, (b * 2 + half) * TB:
                                           (b * 2 + half + 1) * TB].T
    if trace:
        kernel.last_results = res
    return out
